# revision 1
# baseline (speedup 1.0000x reference)
"""AlphaWeightedHead Trainium2 kernel: per-sample sigmoid-gated QKV + MHA + proj.

Sharding: data-parallel over batch, 2 samples per core x 8 cores.
All device tensors use a feature-major ("transposed") layout so every matmul
reads its operands in natural orientation (no on-device transposes):

  x^T [c, t]  -> QKV^T [d, t] (Q/K) and V [t, hv]   (contraction over c)
  S^T [tk,tq] = K^T.T @ Q^T per head (contraction over hd)
  P^T = exp(S^T * scale)  (no max-subtract: |scores| < ~1, data-bounded)
  O^T_aug [65, tq] = [V | ones].T @ P^T  (row 64 = softmax denominator;
      the ones column is folded into the padded V weights host-side)
  Y^T [c_out, t] = pw^T.T @ (O^T / denom)

Perf structure (HW-validated on trn2):
  * QK projection GEMM runs in fp8e4m3 with MatmulPerfMode.DoubleRow:
    contraction 768 = 3 passes x (128 partitions x 2 k-tiles), halving the
    pass count vs bf16. Host pre-scales wqk8 by 16 into fp8's normal range
    and divides the f32 sigmoid gates by 16 to compensate exactly. Q/K/exp
    errors are damped by softmax normalization (rel err ~4e-3 total).
  * S^T stationaries are K=128 zero-padded per head (the other head's 64
    partitions hold zeros, memset once per pool slot). K=64 stationaries
    disable the PE's Fast Weight Load and cost ~2x; zero-padding keeps FWL
    on (-49 us/exec measured vs quadrant-packed K=64 tile_position pairs).
  * Softmax normalize multiplies the PV PSUM directly: reciprocal of the
    denominator row -> gpsimd partition_broadcast (SBUF) -> one DVE
    scalar_tensor_tensor per half-pair. No PSUM->SBUF staging copy and no
    PE broadcast matmul.
  * V GEMM / attention / proj stay bf16: fp8 on the V or proj path passes
    quantization error straight to the output (weighted means don't damp
    multiplicative operand noise), blowing the 2e-2 budget.

  * Pipeline schedule (engine queues are in-order FIFOs, so emission
    order = execution order per engine): the two samples are processed
    as separate pipelines, with each boundary interleaved as
    [V(s)-half0, QK+S^T(s, pair0), V(s)-half1, proj(s-1), pairs 1..5]
    — legal because pair0's PV reads only V columns 0:130. This keeps
    the activation engine fed across the cold start and the sample
    boundary while the previous projection drains under live
    attention. Weight staging (wv) is double-buffered so sample s+1's
    1.2MB DMA prefetches during sample s.
  * Confirmed-optimal by A/B (don't re-litigate): PSUM banks
    (mm 2 / st 4 / pv 2), exp at free-1024, LDWEIGHTS needs no
    stationary reuse, pt/qk/kpp pool depths.

V/PV/proj matmuls run in bf16 (PSUM accumulates fp32). Host precomputes
sigmoid(alpha[label]), pre-scales V weights/biases, and pads V with the
ones column per head (wv zero-col + bias 1.0 -> V-tile column of ones).
`build(reps=N)` unrolls the whole body N times in one NEFF: bench-only
amortization of dispatch noise; grading uses reps=1. Non-default build
flags preserve rejected A/B variants for reference.
"""

import sys

import numpy as np
import ml_dtypes
from contextlib import ExitStack

try:
    import concourse.bass as _probe  # noqa: F401
except ModuleNotFoundError:
    sys.path.insert(0, "/opt/trn_rl_repo")

import concourse.bass as bass
import concourse.bacc as bacc
import concourse.tile as tile
from concourse import mybir
from concourse.bass_utils import run_bass_kernel_spmd

B, NT, C, H, CLS = 16, 1024, 768, 12, 1000
HD = 64
NCORES = 8
SPC = B // NCORES          # samples per core = 2
T = SPC * NT               # tokens per core = 2048
NPAIR = H // 2             # 6 head pairs
CH = C // 128              # 6 contraction chunks
CP = H * (HD + 1)          # padded V width = 780 (65 per head)
SCALE = HD ** -0.5

F32 = mybir.dt.float32
BF16 = mybir.dt.bfloat16
FP8 = mybir.dt.float8e4
W8SCALE = 16.0  # host scales wqk8 by this so weights land in fp8 normals
ADD = mybir.AluOpType.add
MULT = mybir.AluOpType.mult
EXP = mybir.ActivationFunctionType.Exp


def build(debug=False, phases="all", reps=1, st_kpad=True, epi_bcast=True,
          exp_fine=False, split_s=True, pvlag=1, wv2=True, qb=2,
          vhoist=True, vearly=True, vearly2=True, ptslack=False,
          jout=False, pv3=False):
    nc = bacc.Bacc("TRN2")
    xt = nc.declare_dram_parameter("xt", [C, T], BF16, isOutput=False)
    # fp8 DoubleRow operands for the QK projection GEMM: contraction 768 =
    # 3 passes x (128 partitions x 2 k-tiles). Layout [p, pass, ktile, n].
    xt8 = nc.declare_dram_parameter("xt8", [128, 3, 2, T], FP8, isOutput=False)
    wqk8 = nc.declare_dram_parameter("wqk8", [128, 3, 2, 2 * C], FP8,
                                     isOutput=False)
    wv = nc.declare_dram_parameter("wv", [SPC, C, CP], BF16, isOutput=False)
    sigbq = nc.declare_dram_parameter("sigbq", [128, 2 * SPC * 12], F32,
                                      isOutput=False)
    bvs = nc.declare_dram_parameter("bvs", [SPC, CP], BF16, isOutput=False)
    pw = nc.declare_dram_parameter("pw", [C, C], BF16, isOutput=False)
    pb = nc.declare_dram_parameter("pb", [128, CH], F32, isOutput=False)
    out = nc.declare_dram_parameter("out", [C, T], F32, isOutput=True)
    if debug:
        dbg_v = nc.declare_dram_parameter("dbg_v", [128, 8 * CP], F32, isOutput=True)
        dbg_qk = nc.declare_dram_parameter("dbg_qk", [128, 2 * T], F32, isOutput=True)
        dbg_pt = nc.declare_dram_parameter("dbg_pt", [128, 2 * 8 * 512], F32, isOutput=True)
        dbg_st = nc.declare_dram_parameter("dbg_st", [128, 2 * 512], F32, isOutput=True)
        dbg_ob = nc.declare_dram_parameter("dbg_ob", [128, CH * T], F32, isOutput=True)

    with tile.TileContext(nc) as tc, ExitStack() as ctx:
        cpool = ctx.enter_context(tc.tile_pool(name="const", bufs=1))
        wvp = ctx.enter_context(tc.tile_pool(name="wvp", bufs=2 if wv2 else 1))
        qkp = ctx.enter_context(tc.tile_pool(name="qkp", bufs=qb))
        kpp = ctx.enter_context(tc.tile_pool(name="kpp", bufs=qb))
        ptp = ctx.enter_context(tc.tile_pool(name="ptp", bufs=2 * (pvlag + 1) + (2 if ptslack else 0)))
        stgp = ctx.enter_context(tc.tile_pool(name="stgp", bufs=4))
        yp = ctx.enter_context(tc.tile_pool(name="yp", bufs=3))
        mmps = ctx.enter_context(
            tc.tile_pool(name="mmps", bufs=1 if pv3 else 2,
                         space=bass.MemorySpace.PSUM))
        stps = ctx.enter_context(
            tc.tile_pool(name="stps", bufs=4 if exp_fine else 2,
                         space=bass.MemorySpace.PSUM))
        pvps = ctx.enter_context(
            tc.tile_pool(name="pvps", bufs=3 if pv3 else 2,
                         space=bass.MemorySpace.PSUM))

        # ---- resident tensors
        xt_sb = cpool.tile([128, CH, T], BF16)
        for c in range(CH):
            nc.sync.dma_start(xt_sb[:, c, :], xt[c * 128:(c + 1) * 128, :])
        xt8_sb = cpool.tile([128, 3, 2, T], FP8)
        nc.sync.dma_start(xt8_sb[:], xt8[:])
        wqk8_sb = cpool.tile([128, 3, 2, 2 * C], FP8)
        nc.sync.dma_start(wqk8_sb[:], wqk8[:])
        pw_sb = cpool.tile([128, CH, C], BF16)
        nc.sync.dma_start(pw_sb[:], pw.rearrange("(c p) n -> p c n", p=128))
        sigbq_sb = cpool.tile([128, 2 * SPC * 12], F32)
        nc.sync.dma_start(sigbq_sb[:], sigbq[:])
        pb_sb = cpool.tile([128, CH], F32)
        nc.sync.dma_start(pb_sb[:], pb[:])
        bvs_sb = cpool.tile([128, SPC, CP], BF16)
        for s in range(SPC):
            nc.sync.dma_start(
                bvs_sb[:, s:s + 1, :], bvs[s:s + 1, :].partition_broadcast(128))
        ob = cpool.tile([128, CH, T], BF16)
        ones_t = cpool.tile([128, 64], BF16)
        nc.vector.memset(ones_t[0:1, :], 1.0)
        nc.vector.memset(ones_t[64:65, :], 1.0)
        vbs = [cpool.tile([128, 8, CP], BF16, tag=f"vb{s}", name=f"vb{s}")
               for s in range(SPC)]

        # ---- V phase: V_pad[t, 780] = x_s @ wv_pad + bvs_pad  (per sample)
        # wv_pad has a zero column per head; bvs_pad carries 1.0 there, so
        # the padded column becomes the all-ones denominator column.
        # (body emitted `reps` times for bench amortization; reps=1 for
        # grading)

        wv_sbs = {}

        def emit_v_phase(vs=None, hvts=(0, 1)):
          for s in ([vs] if vs is not None else range(SPC)):
            if s in wv_sbs:
                wv_sb = wv_sbs[s]
            else:
                wv_sb = wvp.tile([128, CH, CP], BF16)
                nc.sync.dma_start(
                    wv_sb[:], wv[s].rearrange("(c p) n -> p c n", p=128))
                wv_sbs[s] = wv_sb
            for tt in range(8):
                if jout and len(hvts) == 2:
                    pss = {hvt: mmps.tile([128, 512], F32, tag="mm",
                                          name=f"psv{hvt}")
                           for hvt in hvts}
                    for c in range(CH):
                        for hvt in hvts:
                            h0 = hvt * 512
                            hvn = 512 if hvt == 0 else CP - 512
                            nc.tensor.matmul(
                                pss[hvt][:, :hvn],
                                xt_sb[:, c,
                                      s * NT + tt * 128: s * NT + (tt + 1) * 128],
                                wv_sb[:, c, h0: h0 + hvn],
                                start=(c == 0), stop=(c == CH - 1),
                            )
                    for hvt in hvts:
                        h0 = hvt * 512
                        hvn = 512 if hvt == 0 else CP - 512
                        nc.vector.tensor_add(
                            vbs[s][:, tt, h0:h0 + hvn],
                            pss[hvt][:, :hvn],
                            bvs_sb[:, s, h0:h0 + hvn])
                    continue
                for hvt in hvts:
                    h0 = hvt * 512
                    hvn = 512 if hvt == 0 else CP - 512
                    ps = mmps.tile([128, 512], F32, tag="mm")
                    for c in range(CH):
                        nc.tensor.matmul(
                            ps[:, :hvn],
                            xt_sb[:, c, s * NT + tt * 128: s * NT + (tt + 1) * 128],
                            wv_sb[:, c, h0: h0 + hvn],
                            start=(c == 0), stop=(c == CH - 1),
                        )
                    nc.vector.tensor_add(
                        vbs[s][:, tt, h0:h0 + hvn],
                        ps[:, :hvn],
                        bvs_sb[:, s, h0:h0 + hvn])
                    if debug and s == 0:
                        dv = yp.tile([128, 512], F32, tag="dbgv", name="dv")
                        nc.vector.tensor_copy(dv[:, :hvn], vbs[s][:, tt, h0:h0 + hvn])
                        nc.sync.dma_start(
                            dbg_v[:, tt * CP + h0: tt * CP + h0 + hvn], dv[:, :hvn])

        # ---- PV + epilogue, one iteration behind S^T/exp (keeps ACT fed)
        pending = []

        def emit_pv(item):
            p, s, tq, pt = item
            for hh in range(2):
                h = 2 * p + hh
                pv = pvps.tile([128, 512], F32, tag="pv", name="pv")
                for chk in range(8):
                    nc.tensor.matmul(
                        pv[0:65, :],
                        vbs[s][:, chk, h * 65: h * 65 + 65],
                        pt[hh][:, chk, :],
                        start=(chk == 0), stop=(chk == 7),
                    )
                if epi_bcast:
                    rsb = stgp.tile([1, 512], BF16, tag="rsb", name="rsb")
                    with nc.allow_low_precision(reason="softmax denom bf16"):
                        nc.vector.reciprocal(rsb[:], pv[64:65, :])
                    mul2 = stgp.tile([64, 512], BF16, tag="rbb", name="rbb")
                    nc.gpsimd.partition_broadcast(mul2[:], rsb[:], channels=64)
                    src = pv
                else:
                    stg = stgp.tile([128, 512], BF16, tag="stg", name="stg")
                    nc.vector.tensor_copy(stg[0:65, :], pv[0:65, :])
                    with nc.allow_low_precision(reason="softmax denom bf16"):
                        nc.vector.reciprocal(stg[64:65, :], stg[64:65, :])
                    rb = pvps.tile([128, 512], F32, tag="pv", name="rb")
                    nc.tensor.matmul(
                        rb[0:64, :],
                        ones_t[64:65, :],
                        stg[64:65, :],
                        start=True, stop=True,
                        tile_position=(64, 0),
                    )
                    mul2 = rb
                    src = stg
                if hh == 0:
                    nc.vector.scalar_tensor_tensor(
                        ob[0:64, p, s * NT + tq * 512: s * NT + (tq + 1) * 512],
                        src[0:64, :], 0.0, mul2[0:64, :],
                        mybir.AluOpType.bypass, MULT)
                else:
                    stn = stgp.tile([64, 512], BF16, tag="stn", name="stn")
                    nc.vector.scalar_tensor_tensor(
                        stn[:], src[0:64, :], 0.0, mul2[0:64, :],
                        mybir.AluOpType.bypass, MULT)
                    nc.sync.dma_start(
                        ob[64:128, p,
                           s * NT + tq * 512: s * NT + (tq + 1) * 512],
                        stn[:])

        # ---- head-pair loop: QKV(Q,K) -> S^T -> exp -> PV -> normalize
        # S^T stationaries are K=128 zero-padded per head (the other head's
        # 64 partitions hold zeros) so FWL stays enabled; the moving Q
        # streams both heads' rows and the zeros mask the wrong head.
        # Zero halves are memset once per pool slot and never rewritten.
        TW = NT if split_s else T
        if st_kpad:
            kp_init = []
            for _i in range(2):
                ke = kpp.tile([128, TW], BF16, tag="ke", name="ke")
                ko = kpp.tile([128, TW], BF16, tag="ko", name="ko")
                nc.vector.memset(ke[64:128, :], 0.0)
                nc.vector.memset(ko[0:64, :], 0.0)
                kp_init.append((ke, ko))

        def emit_heads(s_sel=None, pairs=None):
         nos = phases in ("v", "vqk", "qkonly")
         slist = ([] if nos else [s_sel]) if split_s else \
             list(range(0 if nos else SPC))
         if pairs is None:
             pairs = range(NPAIR if phases != "v" else 0)
         for p in pairs:
            qk_t = qkp.tile([128, 1 if st_kpad else 2, TW], BF16)
            if st_kpad:
                ke = kpp.tile([128, TW], BF16, tag="ke", name="ke")
                ko = kpp.tile([128, TW], BF16, tag="ko", name="ko")
            for qk in range(2):
                d0 = qk * C + p * 128
                nlist = list(range(2 if split_s else 4))
                pss = {}
                if jout:
                    for n in nlist:
                        pss[n] = mmps.tile([128, 512], F32, tag="mm",
                                           name=f"psq{n}")
                    for j in range(3):
                        for n in nlist:
                            g = (2 * s_sel + n) if split_s else n
                            nc.tensor.matmul(
                                pss[n][:],
                                wqk8_sb[:, j, :, d0:d0 + 128],
                                xt8_sb[:, j, :, g * 512:(g + 1) * 512],
                                start=(j == 0), stop=(j == 2),
                                perf_mode=mybir.MatmulPerfMode.DoubleRow,
                            )
                for n in nlist:
                    g = (2 * s_sel + n) if split_s else n
                    if jout:
                        ps = pss[n]
                    else:
                        ps = mmps.tile([128, 512], F32, tag="mm")
                        for j in range(3):
                            nc.tensor.matmul(
                                ps[:],
                                wqk8_sb[:, j, :, d0:d0 + 128],
                                xt8_sb[:, j, :, g * 512:(g + 1) * 512],
                                start=(j == 0), stop=(j == 2),
                                perf_mode=mybir.MatmulPerfMode.DoubleRow,
                            )
                    sj = s_sel if split_s else n // 2
                    j = sj * 12 + qk * 6 + p
                    if st_kpad and qk == 1:
                        nc.vector.tensor_scalar(
                            ke[0:64, n * 512:(n + 1) * 512], ps[0:64, :],
                            sigbq_sb[0:64, j:j + 1],
                            sigbq_sb[0:64, 24 + j:24 + j + 1],
                            MULT, ADD)
                        nc.vector.tensor_scalar(
                            ko[64:128, n * 512:(n + 1) * 512], ps[64:128, :],
                            sigbq_sb[64:128, j:j + 1],
                            sigbq_sb[64:128, 24 + j:24 + j + 1],
                            MULT, ADD)
                    else:
                        nc.vector.tensor_scalar(
                            qk_t[:, qk, n * 512:(n + 1) * 512], ps[:],
                            sigbq_sb[:, j:j + 1], sigbq_sb[:, 24 + j:24 + j + 1],
                            MULT, ADD)
                    if debug and p == 0 and not st_kpad:
                        dq = yp.tile([128, 512], F32, tag="dbgq", name="dq")
                        nc.vector.tensor_copy(
                            dq[:], qk_t[:, qk, n * 512:(n + 1) * 512])
                        nc.sync.dma_start(
                            dbg_qk[:, qk * T + n * 512: qk * T + (n + 1) * 512],
                            dq[:])

            for s in slist:
                base = 0 if split_s else s * NT
                for tq in range(2):
                    pt = [ptp.tile([128, 8, 512], BF16, tag="pt", name=f"pt{_h}")
                          for _h in range(2)]

                    def st_mm(dst, tk, hh):
                        lo = hh * 64
                        if st_kpad:
                            kt = ko if hh else ke
                            nc.tensor.matmul(
                                dst,
                                kt[:, base + tk * 128:
                                   base + (tk + 1) * 128],
                                qk_t[:, 0,
                                     base + tq * 512:
                                     base + (tq + 1) * 512],
                                start=True, stop=True,
                            )
                        else:
                            nc.tensor.matmul(
                                dst,
                                qk_t[lo:lo + 64, 1,
                                     base + tk * 128:
                                     base + (tk + 1) * 128],
                                qk_t[lo:lo + 64, 0,
                                     base + tq * 512:
                                     base + (tq + 1) * 512],
                                start=True, stop=True,
                                tile_position=(lo, 0),
                            )

                    if exp_fine:
                        for tk in range(8):
                            stf = [stps.tile([128, 512], F32, tag="st",
                                             name=f"stf{_h}")
                                   for _h in range(2)]
                            for hh in range(2):
                                st_mm(stf[hh][:], tk, hh)
                            for hh in range(2):
                                nc.scalar.activation(
                                    pt[hh][:, tk, :],
                                    stf[hh][:], EXP, scale=SCALE)
                    else:
                        for tk2 in range(4):
                            st2 = [stps.tile([128, 2, 512], F32, tag="st",
                                             name=f"st{_h}")
                                   for _h in range(2)]
                            for sub in range(2):
                                tk = 2 * tk2 + sub
                                for hh in range(2):
                                    st_mm(st2[hh][:, sub, :], tk, hh)
                            for hh in range(2):
                                nc.scalar.activation(
                                    pt[hh][:, 2 * tk2:2 * tk2 + 2, :],
                                    st2[hh][:], EXP, scale=SCALE)
                    if phases != "stexp":
                        pending.append((p, s, tq, pt))
                        if len(pending) > pvlag:
                            emit_pv(pending.pop(0))
        def emit_tail(nlist):
          if debug and phases == "all":
            for c in range(CH):
                for n in nlist:
                    do = yp.tile([128, 512], F32, tag="dbgo", name="do")
                    nc.vector.tensor_copy(do[:], ob[:, c, n * 512:(n + 1) * 512])
                    nc.sync.dma_start(
                        dbg_ob[:, c * T + n * 512: c * T + (n + 1) * 512], do[:])
          # ---- proj: Y^T = pw^T.T @ O^T + pb
          for m in range(CH if phases == "all" else 0):
            for n in nlist:
                ps = mmps.tile([128, 512], F32, tag="mm")
                for c in range(CH):
                    nc.tensor.matmul(
                        ps[:],
                        pw_sb[:, c, m * 128:(m + 1) * 128],
                        ob[:, c, n * 512:(n + 1) * 512],
                        start=(c == 0), stop=(c == CH - 1),
                    )
                y_t = yp.tile([128, 512], F32)
                nc.vector.tensor_scalar(y_t[:], ps[:], pb_sb[:, m:m + 1], None, ADD)
                nc.sync.dma_start(
                    out[m * 128:(m + 1) * 128, n * 512:(n + 1) * 512], y_t[:])

        for _rep in range(reps):
            if split_s and vearly2 and phases == "all":
                # interleave each sample's V/attention start with the
                # previous sample's projection to keep ACT fed across
                # the boundary (PE queues are in-order FIFOs)
                for s in range(SPC):
                    emit_v_phase(s, hvts=(0,))
                    emit_heads(s, pairs=[0])
                    emit_v_phase(s, hvts=(1,))
                    if s > 0:
                        emit_tail([2 * (s - 1), 2 * (s - 1) + 1])
                    emit_heads(s, pairs=list(range(1, NPAIR)))
                    while pending:
                        emit_pv(pending.pop(0))
                emit_tail([2 * (SPC - 1), 2 * (SPC - 1) + 1])
            elif split_s:
                for s in range(SPC):
                    if s == 0:
                        if vearly and phases == "all":
                            emit_v_phase(0, hvts=(0,))
                            emit_heads(0, pairs=[0])
                            emit_v_phase(0, hvts=(1,))
                            emit_heads(0, pairs=list(range(1, NPAIR)))
                        else:
                            emit_v_phase(0)
                            emit_heads(0)
                    else:
                        emit_heads(s)
                    while pending:
                        emit_pv(pending.pop(0))
                    if vhoist and s + 1 < SPC:
                        emit_v_phase(s + 1)
                    emit_tail([2 * s, 2 * s + 1])
                    if not vhoist and s + 1 < SPC:
                        emit_v_phase(s + 1)
            else:
                emit_v_phase()
            if not split_s:
                emit_heads()
                while pending:
                    emit_pv(pending.pop(0))
                emit_tail([0, 1, 2, 3])
    nc.compile()
    return nc


def make_in_maps(x, label, alpha, qkv_w, qkv_b, proj_w, proj_b):
    x = np.asarray(x, np.float32)
    label = np.asarray(label)
    alpha = np.asarray(alpha, np.float32)
    qkv_w = np.asarray(qkv_w, np.float32)
    qkv_b = np.asarray(qkv_b, np.float32)
    proj_w = np.asarray(proj_w, np.float32)
    proj_b = np.asarray(proj_b, np.float32)

    sig = 1.0 / (1.0 + np.exp(-alpha[label]))          # (B, 3C) f32
    wqkT = np.ascontiguousarray(qkv_w[:2 * C].T)        # (C, 2C) f32
    # fp8 DoubleRow layout [p, pass, ktile, d], weights pre-scaled by
    # W8SCALE into fp8's normal range (compensated in the sig scalars)
    wqk8 = np.ascontiguousarray(
        (wqkT * W8SCALE).reshape(3, 2, 128, 2 * C).transpose(2, 0, 1, 3)
    ).astype(ml_dtypes.float8_e4m3)
    wvT = np.ascontiguousarray(qkv_w[2 * C:].T)         # (C, C) f32
    pw_bf = np.ascontiguousarray(proj_w.T).astype(ml_dtypes.bfloat16)
    pb_arr = np.ascontiguousarray(proj_b.reshape(CH, 128).T)

    in_maps = []
    for i in range(NCORES):
        sl = slice(SPC * i, SPC * (i + 1))
        xs = x[sl]                                      # (2, NT, C)
        xt_f = xs.transpose(2, 0, 1).reshape(C, T)      # (C, T) f32
        xt = np.ascontiguousarray(xt_f).astype(ml_dtypes.bfloat16)
        xt8 = np.ascontiguousarray(
            xt_f.reshape(3, 2, 128, T).transpose(2, 0, 1, 3)
        ).astype(ml_dtypes.float8_e4m3)
        sig_i = sig[sl]                                 # (2, 3C)
        sqk = sig_i[:, :2 * C]                          # (2, 2C)
        sq = ((sqk / W8SCALE).reshape(SPC, 12, 128)
              .transpose(2, 0, 1).reshape(128, SPC * 12))
        bq = ((qkv_b[None, :2 * C] * sqk).reshape(SPC, 12, 128)
              .transpose(2, 0, 1).reshape(128, SPC * 12))
        sigbq_i = np.ascontiguousarray(np.concatenate([sq, bq], axis=1))
        sigv = sig_i[:, 2 * C:]                         # (2, C)
        wv_sc = wvT[None, :, :] * sigv[:, None, :]      # (2, C, C)
        wv_pad = np.zeros((SPC, C, CP), np.float32)
        bvs_pad = np.zeros((SPC, CP), np.float32)
        for h in range(H):
            wv_pad[:, :, h * 65:h * 65 + 64] = wv_sc[:, :, h * 64:(h + 1) * 64]
            bvs_pad[:, h * 65:h * 65 + 64] = (
                qkv_b[None, 2 * C + h * 64: 2 * C + (h + 1) * 64]
                * sigv[:, h * 64:(h + 1) * 64])
            bvs_pad[:, h * 65 + 64] = 1.0
        in_maps.append({
            "xt": xt, "xt8": xt8, "wqk8": wqk8,
            "wv": np.ascontiguousarray(wv_pad).astype(ml_dtypes.bfloat16),
            "sigbq": sigbq_i,
            "bvs": np.ascontiguousarray(bvs_pad).astype(ml_dtypes.bfloat16),
            "pw": pw_bf, "pb": pb_arr,
        })
    return in_maps


_NC = None
LAST_RESULT = None


def kernel(x, label, alpha, qkv_w, qkv_b, proj_w, proj_b):
    global _NC, LAST_RESULT
    if _NC is None:
        _NC = build()
    in_maps = make_in_maps(x, label, alpha, qkv_w, qkv_b, proj_w, proj_b)
    res = run_bass_kernel_spmd(_NC, in_maps, core_ids=list(range(NCORES)))
    LAST_RESULT = res
    outs = []
    for i in range(NCORES):
        y = np.asarray(res.results[i]["out"])           # (C, T)
        outs.append(y.reshape(C, SPC, NT).transpose(1, 2, 0))
    return np.ascontiguousarray(np.concatenate(outs, axis=0), dtype=np.float32)



# revision 5
# speedup vs baseline: 1.0052x; 1.0052x over previous
"""AlphaWeightedHead Trainium2 kernel: per-sample sigmoid-gated QKV + MHA + proj.

Sharding: data-parallel over batch, 2 samples per core x 8 cores.
All device tensors use a feature-major ("transposed") layout so every matmul
reads its operands in natural orientation (no on-device transposes):

  x^T [c, t]  -> QKV^T [d, t] (Q/K) and V [t, hv]   (contraction over c)
  S^T [tk,tq] = K^T.T @ Q^T per head (contraction over hd)
  P^T = exp(S^T * scale)  (no max-subtract: |scores| < ~1, data-bounded)
  O^T_aug [65, tq] = [V | ones].T @ P^T  (row 64 = softmax denominator;
      the ones column is folded into the padded V weights host-side)
  Y^T [c_out, t] = pw^T.T @ (O^T / denom)

Perf structure (HW-validated on trn2):
  * QK projection GEMM runs in fp8e4m3 with MatmulPerfMode.DoubleRow:
    contraction 768 = 3 passes x (128 partitions x 2 k-tiles), halving the
    pass count vs bf16. Host pre-scales wqk8 by 16 into fp8's normal range
    and divides the f32 sigmoid gates by 16 to compensate exactly. Q/K/exp
    errors are damped by softmax normalization (rel err ~4e-3 total).
  * S^T stationaries are K=128 zero-padded per head (the other head's 64
    partitions hold zeros, memset once per pool slot). K=64 stationaries
    disable the PE's Fast Weight Load and cost ~2x; zero-padding keeps FWL
    on (-49 us/exec measured vs quadrant-packed K=64 tile_position pairs).
  * Softmax normalize multiplies the PV PSUM directly: reciprocal of the
    denominator row -> gpsimd partition_broadcast (SBUF) -> one DVE
    scalar_tensor_tensor per half-pair. No PSUM->SBUF staging copy and no
    PE broadcast matmul.
  * V GEMM / attention / proj stay bf16: fp8 on the V or proj path passes
    quantization error straight to the output (weighted means don't damp
    multiplicative operand noise), blowing the 2e-2 budget.

  * Pipeline schedule (engine queues are in-order FIFOs, so emission
    order = execution order per engine): the two samples are processed
    as separate pipelines, with each boundary interleaved as
    [V(s)-half0, QK+S^T(s, pair0), V(s)-half1, proj(s-1), pairs 1..5]
    — legal because pair0's PV reads only V columns 0:130. This keeps
    the activation engine fed across the cold start and the sample
    boundary while the previous projection drains under live
    attention. Weight staging (wv) is double-buffered so sample s+1's
    1.2MB DMA prefetches during sample s.
  * Confirmed-optimal by A/B (don't re-litigate): PSUM banks
    (mm 2 / st 4 / pv 2), exp at free-1024, LDWEIGHTS needs no
    stationary reuse, pt/qk/kpp pool depths.

V/PV/proj matmuls run in bf16 (PSUM accumulates fp32). Host precomputes
sigmoid(alpha[label]), pre-scales V weights/biases, and pads V with the
ones column per head (wv zero-col + bias 1.0 -> V-tile column of ones).
`build(reps=N)` unrolls the whole body N times in one NEFF: bench-only
amortization of dispatch noise; grading uses reps=1. Non-default build
flags preserve rejected A/B variants for reference.
"""

import sys

import numpy as np
import ml_dtypes
from contextlib import ExitStack

try:
    import concourse.bass as _probe  # noqa: F401
except ModuleNotFoundError:
    sys.path.insert(0, "/opt/trn_rl_repo")

import concourse.bass as bass
import concourse.bacc as bacc
import concourse.tile as tile
from concourse import mybir
from concourse.bass_utils import run_bass_kernel_spmd

B, NT, C, H, CLS = 16, 1024, 768, 12, 1000
HD = 64
NCORES = 8
SPC = B // NCORES          # samples per core = 2
T = SPC * NT               # tokens per core = 2048
NPAIR = H // 2             # 6 head pairs
CH = C // 128              # 6 contraction chunks
CP = H * (HD + 1)          # padded V width = 780 (65 per head)
SCALE = HD ** -0.5

F32 = mybir.dt.float32
BF16 = mybir.dt.bfloat16
FP8 = mybir.dt.float8e4
W8SCALE = 16.0  # host scales wqk8 by this so weights land in fp8 normals
ADD = mybir.AluOpType.add
MULT = mybir.AluOpType.mult
EXP = mybir.ActivationFunctionType.Exp


def build(debug=False, phases="all", reps=1, st_kpad=True, epi_bcast=True,
          exp_fine=False, split_s=True, pvlag=1, wv2=True, qb=2,
          vhoist=True, vearly=True, vearly2=True, ptslack=False,
          jout=False, pv3=False, pvflip=True):
    nc = bacc.Bacc("TRN2")
    xt = nc.declare_dram_parameter("xt", [C, T], BF16, isOutput=False)
    # fp8 DoubleRow operands for the QK projection GEMM: contraction 768 =
    # 3 passes x (128 partitions x 2 k-tiles). Layout [p, pass, ktile, n].
    xt8 = nc.declare_dram_parameter("xt8", [128, 3, 2, T], FP8, isOutput=False)
    wqk8 = nc.declare_dram_parameter("wqk8", [128, 3, 2, 2 * C], FP8,
                                     isOutput=False)
    wv = nc.declare_dram_parameter("wv", [SPC, C, CP], BF16, isOutput=False)
    sigbq = nc.declare_dram_parameter("sigbq", [128, 2 * SPC * 12], F32,
                                      isOutput=False)
    bvs = nc.declare_dram_parameter("bvs", [SPC, CP], BF16, isOutput=False)
    pw = nc.declare_dram_parameter("pw", [C, C], BF16, isOutput=False)
    pb = nc.declare_dram_parameter("pb", [128, CH], F32, isOutput=False)
    out = nc.declare_dram_parameter("out", [C, T], F32, isOutput=True)
    if debug:
        dbg_v = nc.declare_dram_parameter("dbg_v", [128, 8 * CP], F32, isOutput=True)
        dbg_qk = nc.declare_dram_parameter("dbg_qk", [128, 2 * T], F32, isOutput=True)
        dbg_pt = nc.declare_dram_parameter("dbg_pt", [128, 2 * 8 * 512], F32, isOutput=True)
        dbg_st = nc.declare_dram_parameter("dbg_st", [128, 2 * 512], F32, isOutput=True)
        dbg_ob = nc.declare_dram_parameter("dbg_ob", [128, CH * T], F32, isOutput=True)

    with tile.TileContext(nc) as tc, ExitStack() as ctx:
        cpool = ctx.enter_context(tc.tile_pool(name="const", bufs=1))
        wvp = ctx.enter_context(tc.tile_pool(name="wvp", bufs=2 if wv2 else 1))
        qkp = ctx.enter_context(tc.tile_pool(name="qkp", bufs=qb))
        kpp = ctx.enter_context(tc.tile_pool(name="kpp", bufs=qb))
        ptp = ctx.enter_context(tc.tile_pool(name="ptp", bufs=2 * (pvlag + 1) + (2 if ptslack else 0)))
        stgp = ctx.enter_context(tc.tile_pool(name="stgp", bufs=4))
        yp = ctx.enter_context(tc.tile_pool(name="yp", bufs=3))
        mmps = ctx.enter_context(
            tc.tile_pool(name="mmps", bufs=1 if pv3 else 2,
                         space=bass.MemorySpace.PSUM))
        stps = ctx.enter_context(
            tc.tile_pool(name="stps", bufs=4 if exp_fine else 2,
                         space=bass.MemorySpace.PSUM))
        pvps = ctx.enter_context(
            tc.tile_pool(name="pvps", bufs=3 if pv3 else 2,
                         space=bass.MemorySpace.PSUM))

        # ---- resident tensors
        xt_sb = cpool.tile([128, CH, T], BF16)
        for c in range(CH):
            nc.sync.dma_start(xt_sb[:, c, :], xt[c * 128:(c + 1) * 128, :])
        xt8_sb = cpool.tile([128, 3, 2, T], FP8)
        nc.sync.dma_start(xt8_sb[:], xt8[:])
        wqk8_sb = cpool.tile([128, 3, 2, 2 * C], FP8)
        nc.sync.dma_start(wqk8_sb[:], wqk8[:])
        pw_sb = cpool.tile([128, CH, C], BF16)
        nc.sync.dma_start(pw_sb[:], pw.rearrange("(c p) n -> p c n", p=128))
        sigbq_sb = cpool.tile([128, 2 * SPC * 12], F32)
        nc.sync.dma_start(sigbq_sb[:], sigbq[:])
        pb_sb = cpool.tile([128, CH], F32)
        nc.sync.dma_start(pb_sb[:], pb[:])
        bvs_sb = cpool.tile([128, SPC, CP], BF16)
        for s in range(SPC):
            nc.sync.dma_start(
                bvs_sb[:, s:s + 1, :], bvs[s:s + 1, :].partition_broadcast(128))
        # pvflip: ob is t-chunk-major [tq-part? no: c-part, tchunk, cchunk(=pair),
        # 128 t] so each DMA-transposed [128tq, 128c] tile lands as one
        # contiguous 256B run per partition (XBAR transpose needs contiguous
        # SBUF dest runs).
        if pvflip:
            ob = cpool.tile([128, T // 128, CH, 128], BF16)
        else:
            ob = cpool.tile([128, CH, T], BF16)
        ones_t = cpool.tile([128, 64], BF16)
        nc.vector.memset(ones_t[0:1, :], 1.0)
        nc.vector.memset(ones_t[64:65, :], 1.0)
        vbs = [cpool.tile([128, 8, CP], BF16, tag=f"vb{s}", name=f"vb{s}")
               for s in range(SPC)]

        # ---- V phase: V_pad[t, 780] = x_s @ wv_pad + bvs_pad  (per sample)
        # wv_pad has a zero column per head; bvs_pad carries 1.0 there, so
        # the padded column becomes the all-ones denominator column.
        # (body emitted `reps` times for bench amortization; reps=1 for
        # grading)

        wv_sbs = {}

        def emit_v_phase(vs=None, hvts=(0, 1)):
          for s in ([vs] if vs is not None else range(SPC)):
            if s in wv_sbs:
                wv_sb = wv_sbs[s]
            else:
                wv_sb = wvp.tile([128, CH, CP], BF16)
                nc.sync.dma_start(
                    wv_sb[:], wv[s].rearrange("(c p) n -> p c n", p=128))
                wv_sbs[s] = wv_sb
            for tt in range(8):
                if jout and len(hvts) == 2:
                    pss = {hvt: mmps.tile([128, 512], F32, tag="mm",
                                          name=f"psv{hvt}")
                           for hvt in hvts}
                    for c in range(CH):
                        for hvt in hvts:
                            h0 = hvt * 512
                            hvn = 512 if hvt == 0 else CP - 512
                            nc.tensor.matmul(
                                pss[hvt][:, :hvn],
                                xt_sb[:, c,
                                      s * NT + tt * 128: s * NT + (tt + 1) * 128],
                                wv_sb[:, c, h0: h0 + hvn],
                                start=(c == 0), stop=(c == CH - 1),
                            )
                    for hvt in hvts:
                        h0 = hvt * 512
                        hvn = 512 if hvt == 0 else CP - 512
                        nc.vector.tensor_add(
                            vbs[s][:, tt, h0:h0 + hvn],
                            pss[hvt][:, :hvn],
                            bvs_sb[:, s, h0:h0 + hvn])
                    continue
                for hvt in hvts:
                    h0 = hvt * 512
                    hvn = 512 if hvt == 0 else CP - 512
                    ps = mmps.tile([128, 512], F32, tag="mm")
                    for c in range(CH):
                        nc.tensor.matmul(
                            ps[:, :hvn],
                            xt_sb[:, c, s * NT + tt * 128: s * NT + (tt + 1) * 128],
                            wv_sb[:, c, h0: h0 + hvn],
                            start=(c == 0), stop=(c == CH - 1),
                        )
                    nc.vector.tensor_add(
                        vbs[s][:, tt, h0:h0 + hvn],
                        ps[:, :hvn],
                        bvs_sb[:, s, h0:h0 + hvn])
                    if debug and s == 0:
                        dv = yp.tile([128, 512], F32, tag="dbgv", name="dv")
                        nc.vector.tensor_copy(dv[:, :hvn], vbs[s][:, tt, h0:h0 + hvn])
                        nc.sync.dma_start(
                            dbg_v[:, tt * CP + h0: tt * CP + h0 + hvn], dv[:, :hvn])

        # ---- PV + epilogue, one iteration behind S^T/exp (keeps ACT fed)
        pending = []

        def emit_pv_flip(item):
            # P^T-stationary PV: out O[tq128, 65] uses all 128 PSUM rows
            # (vs 65 with V-stationary), halving PV's PE cycles. The
            # normalized [tq, c-pair] tile is transposed into ob by the
            # DMA XBAR (14ns per 16x128 tile, zero PE cost).
            p, s, tq, pt = item
            for sub in range(4):
                q0 = sub * 128
                tt = (s * NT + tq * 512) // 128 + sub
                pv = pvps.tile([128, 2, 65], F32, tag="pv", name="pv")
                for hh in range(2):
                    for chk in range(8):
                        nc.tensor.matmul(
                            pv[:, hh, :],
                            pt[hh][:, chk, q0:q0 + 128],
                            vbs[s][:, chk, (2 * p + hh) * 65:
                                   (2 * p + hh) * 65 + 65],
                            start=(chk == 0), stop=(chk == 7),
                        )
                rsb = stgp.tile([128, 2], F32, tag="rsb", name="rsb")
                otok = stgp.tile([128, 128], BF16, tag="otok", name="otok")
                for hh in range(2):
                    nc.vector.reciprocal(
                        rsb[:, hh:hh + 1], pv[:, hh, 64:65])
                    nc.vector.tensor_scalar(
                        otok[:, hh * 64:(hh + 1) * 64], pv[:, hh, 0:64],
                        rsb[:, hh:hh + 1], None, MULT)
                nc.sync.dma_start(ob[:, tt, p, :], otok[:], transpose=True)

        def emit_pv(item):
            if pvflip:
                emit_pv_flip(item)
                return
            p, s, tq, pt = item
            for hh in range(2):
                h = 2 * p + hh
                pv = pvps.tile([128, 512], F32, tag="pv", name="pv")
                for chk in range(8):
                    nc.tensor.matmul(
                        pv[0:65, :],
                        vbs[s][:, chk, h * 65: h * 65 + 65],
                        pt[hh][:, chk, :],
                        start=(chk == 0), stop=(chk == 7),
                    )
                if epi_bcast:
                    rsb = stgp.tile([1, 512], BF16, tag="rsb", name="rsb")
                    with nc.allow_low_precision(reason="softmax denom bf16"):
                        nc.vector.reciprocal(rsb[:], pv[64:65, :])
                    mul2 = stgp.tile([64, 512], BF16, tag="rbb", name="rbb")
                    nc.gpsimd.partition_broadcast(mul2[:], rsb[:], channels=64)
                    src = pv
                else:
                    stg = stgp.tile([128, 512], BF16, tag="stg", name="stg")
                    nc.vector.tensor_copy(stg[0:65, :], pv[0:65, :])
                    with nc.allow_low_precision(reason="softmax denom bf16"):
                        nc.vector.reciprocal(stg[64:65, :], stg[64:65, :])
                    rb = pvps.tile([128, 512], F32, tag="pv", name="rb")
                    nc.tensor.matmul(
                        rb[0:64, :],
                        ones_t[64:65, :],
                        stg[64:65, :],
                        start=True, stop=True,
                        tile_position=(64, 0),
                    )
                    mul2 = rb
                    src = stg
                if hh == 0:
                    nc.vector.scalar_tensor_tensor(
                        ob[0:64, p, s * NT + tq * 512: s * NT + (tq + 1) * 512],
                        src[0:64, :], 0.0, mul2[0:64, :],
                        mybir.AluOpType.bypass, MULT)
                else:
                    stn = stgp.tile([64, 512], BF16, tag="stn", name="stn")
                    nc.vector.scalar_tensor_tensor(
                        stn[:], src[0:64, :], 0.0, mul2[0:64, :],
                        mybir.AluOpType.bypass, MULT)
                    nc.sync.dma_start(
                        ob[64:128, p,
                           s * NT + tq * 512: s * NT + (tq + 1) * 512],
                        stn[:])

        # ---- head-pair loop: QKV(Q,K) -> S^T -> exp -> PV -> normalize
        # S^T stationaries are K=128 zero-padded per head (the other head's
        # 64 partitions hold zeros) so FWL stays enabled; the moving Q
        # streams both heads' rows and the zeros mask the wrong head.
        # Zero halves are memset once per pool slot and never rewritten.
        TW = NT if split_s else T
        if st_kpad:
            kp_init = []
            for _i in range(2):
                ke = kpp.tile([128, TW], BF16, tag="ke", name="ke")
                ko = kpp.tile([128, TW], BF16, tag="ko", name="ko")
                nc.vector.memset(ke[64:128, :], 0.0)
                nc.vector.memset(ko[0:64, :], 0.0)
                kp_init.append((ke, ko))

        def emit_heads(s_sel=None, pairs=None):
         nos = phases in ("v", "vqk", "qkonly")
         slist = ([] if nos else [s_sel]) if split_s else \
             list(range(0 if nos else SPC))
         if pairs is None:
             pairs = range(NPAIR if phases != "v" else 0)
         for p in pairs:
            qk_t = qkp.tile([128, 1 if st_kpad else 2, TW], BF16)
            if st_kpad:
                ke = kpp.tile([128, TW], BF16, tag="ke", name="ke")
                ko = kpp.tile([128, TW], BF16, tag="ko", name="ko")
            for qk in range(2):
                d0 = qk * C + p * 128
                nlist = list(range(2 if split_s else 4))
                pss = {}
                if jout:
                    for n in nlist:
                        pss[n] = mmps.tile([128, 512], F32, tag="mm",
                                           name=f"psq{n}")
                    for j in range(3):
                        for n in nlist:
                            g = (2 * s_sel + n) if split_s else n
                            nc.tensor.matmul(
                                pss[n][:],
                                wqk8_sb[:, j, :, d0:d0 + 128],
                                xt8_sb[:, j, :, g * 512:(g + 1) * 512],
                                start=(j == 0), stop=(j == 2),
                                perf_mode=mybir.MatmulPerfMode.DoubleRow,
                            )
                for n in nlist:
                    g = (2 * s_sel + n) if split_s else n
                    if jout:
                        ps = pss[n]
                    else:
                        ps = mmps.tile([128, 512], F32, tag="mm")
                        for j in range(3):
                            nc.tensor.matmul(
                                ps[:],
                                wqk8_sb[:, j, :, d0:d0 + 128],
                                xt8_sb[:, j, :, g * 512:(g + 1) * 512],
                                start=(j == 0), stop=(j == 2),
                                perf_mode=mybir.MatmulPerfMode.DoubleRow,
                            )
                    sj = s_sel if split_s else n // 2
                    j = sj * 12 + qk * 6 + p
                    if st_kpad and qk == 1:
                        nc.vector.tensor_scalar(
                            ke[0:64, n * 512:(n + 1) * 512], ps[0:64, :],
                            sigbq_sb[0:64, j:j + 1],
                            sigbq_sb[0:64, 24 + j:24 + j + 1],
                            MULT, ADD)
                        nc.vector.tensor_scalar(
                            ko[64:128, n * 512:(n + 1) * 512], ps[64:128, :],
                            sigbq_sb[64:128, j:j + 1],
                            sigbq_sb[64:128, 24 + j:24 + j + 1],
                            MULT, ADD)
                    else:
                        nc.vector.tensor_scalar(
                            qk_t[:, qk, n * 512:(n + 1) * 512], ps[:],
                            sigbq_sb[:, j:j + 1], sigbq_sb[:, 24 + j:24 + j + 1],
                            MULT, ADD)
                    if debug and p == 0 and not st_kpad:
                        dq = yp.tile([128, 512], F32, tag="dbgq", name="dq")
                        nc.vector.tensor_copy(
                            dq[:], qk_t[:, qk, n * 512:(n + 1) * 512])
                        nc.sync.dma_start(
                            dbg_qk[:, qk * T + n * 512: qk * T + (n + 1) * 512],
                            dq[:])

            for s in slist:
                base = 0 if split_s else s * NT
                for tq in range(2):
                    pt = [ptp.tile([128, 8, 512], BF16, tag="pt", name=f"pt{_h}")
                          for _h in range(2)]

                    def st_mm(dst, tk, hh):
                        lo = hh * 64
                        if st_kpad:
                            kt = ko if hh else ke
                            nc.tensor.matmul(
                                dst,
                                kt[:, base + tk * 128:
                                   base + (tk + 1) * 128],
                                qk_t[:, 0,
                                     base + tq * 512:
                                     base + (tq + 1) * 512],
                                start=True, stop=True,
                            )
                        else:
                            nc.tensor.matmul(
                                dst,
                                qk_t[lo:lo + 64, 1,
                                     base + tk * 128:
                                     base + (tk + 1) * 128],
                                qk_t[lo:lo + 64, 0,
                                     base + tq * 512:
                                     base + (tq + 1) * 512],
                                start=True, stop=True,
                                tile_position=(lo, 0),
                            )

                    if exp_fine:
                        for tk in range(8):
                            stf = [stps.tile([128, 512], F32, tag="st",
                                             name=f"stf{_h}")
                                   for _h in range(2)]
                            for hh in range(2):
                                st_mm(stf[hh][:], tk, hh)
                            for hh in range(2):
                                nc.scalar.activation(
                                    pt[hh][:, tk, :],
                                    stf[hh][:], EXP, scale=SCALE)
                    else:
                        for tk2 in range(4):
                            st2 = [stps.tile([128, 2, 512], F32, tag="st",
                                             name=f"st{_h}")
                                   for _h in range(2)]
                            for sub in range(2):
                                tk = 2 * tk2 + sub
                                for hh in range(2):
                                    st_mm(st2[hh][:, sub, :], tk, hh)
                            for hh in range(2):
                                nc.scalar.activation(
                                    pt[hh][:, 2 * tk2:2 * tk2 + 2, :],
                                    st2[hh][:], EXP, scale=SCALE)
                    if phases != "stexp":
                        pending.append((p, s, tq, pt))
                        if len(pending) > pvlag:
                            emit_pv(pending.pop(0))
        def ob_mov(c, n):
            # O^T moving slice [128c, 512t] for proj: 4 t-chunks of 128
            if pvflip:
                return ob[:, 4 * n:4 * n + 4, c, :]
            return ob[:, c, n * 512:(n + 1) * 512]

        def emit_tail(nlist):
          if debug and phases == "all":
            for c in range(CH):
                for n in nlist:
                    do = yp.tile([128, 512], F32, tag="dbgo", name="do")
                    nc.vector.tensor_copy(do[:], ob_mov(c, n))
                    nc.sync.dma_start(
                        dbg_ob[:, c * T + n * 512: c * T + (n + 1) * 512], do[:])
          # ---- proj: Y^T = pw^T.T @ O^T + pb
          for m in range(CH if phases == "all" else 0):
            for n in nlist:
                ps = mmps.tile([128, 512], F32, tag="mm")
                for c in range(CH):
                    nc.tensor.matmul(
                        ps[:],
                        pw_sb[:, c, m * 128:(m + 1) * 128],
                        ob_mov(c, n),
                        start=(c == 0), stop=(c == CH - 1),
                    )
                y_t = yp.tile([128, 512], F32)
                nc.vector.tensor_scalar(y_t[:], ps[:], pb_sb[:, m:m + 1], None, ADD)
                nc.sync.dma_start(
                    out[m * 128:(m + 1) * 128, n * 512:(n + 1) * 512], y_t[:])

        for _rep in range(reps):
            if split_s and vearly2 and phases == "all":
                # interleave each sample's V/attention start with the
                # previous sample's projection to keep ACT fed across
                # the boundary (PE queues are in-order FIFOs)
                for s in range(SPC):
                    emit_v_phase(s, hvts=(0,))
                    emit_heads(s, pairs=[0])
                    emit_v_phase(s, hvts=(1,))
                    if s > 0:
                        emit_tail([2 * (s - 1), 2 * (s - 1) + 1])
                    emit_heads(s, pairs=list(range(1, NPAIR)))
                    while pending:
                        emit_pv(pending.pop(0))
                emit_tail([2 * (SPC - 1), 2 * (SPC - 1) + 1])
            elif split_s:
                for s in range(SPC):
                    if s == 0:
                        if vearly and phases == "all":
                            emit_v_phase(0, hvts=(0,))
                            emit_heads(0, pairs=[0])
                            emit_v_phase(0, hvts=(1,))
                            emit_heads(0, pairs=list(range(1, NPAIR)))
                        else:
                            emit_v_phase(0)
                            emit_heads(0)
                    else:
                        emit_heads(s)
                    while pending:
                        emit_pv(pending.pop(0))
                    if vhoist and s + 1 < SPC:
                        emit_v_phase(s + 1)
                    emit_tail([2 * s, 2 * s + 1])
                    if not vhoist and s + 1 < SPC:
                        emit_v_phase(s + 1)
            else:
                emit_v_phase()
            if not split_s:
                emit_heads()
                while pending:
                    emit_pv(pending.pop(0))
                emit_tail([0, 1, 2, 3])
    nc.compile()
    return nc


def make_in_maps(x, label, alpha, qkv_w, qkv_b, proj_w, proj_b):
    x = np.asarray(x, np.float32)
    label = np.asarray(label)
    alpha = np.asarray(alpha, np.float32)
    qkv_w = np.asarray(qkv_w, np.float32)
    qkv_b = np.asarray(qkv_b, np.float32)
    proj_w = np.asarray(proj_w, np.float32)
    proj_b = np.asarray(proj_b, np.float32)

    sig = 1.0 / (1.0 + np.exp(-alpha[label]))          # (B, 3C) f32
    wqkT = np.ascontiguousarray(qkv_w[:2 * C].T)        # (C, 2C) f32
    # fp8 DoubleRow layout [p, pass, ktile, d], weights pre-scaled by
    # W8SCALE into fp8's normal range (compensated in the sig scalars)
    wqk8 = np.ascontiguousarray(
        (wqkT * W8SCALE).reshape(3, 2, 128, 2 * C).transpose(2, 0, 1, 3)
    ).astype(ml_dtypes.float8_e4m3)
    wvT = np.ascontiguousarray(qkv_w[2 * C:].T)         # (C, C) f32
    pw_bf = np.ascontiguousarray(proj_w.T).astype(ml_dtypes.bfloat16)
    pb_arr = np.ascontiguousarray(proj_b.reshape(CH, 128).T)

    in_maps = []
    for i in range(NCORES):
        sl = slice(SPC * i, SPC * (i + 1))
        xs = x[sl]                                      # (2, NT, C)
        xt_f = xs.transpose(2, 0, 1).reshape(C, T)      # (C, T) f32
        xt = np.ascontiguousarray(xt_f).astype(ml_dtypes.bfloat16)
        xt8 = np.ascontiguousarray(
            xt_f.reshape(3, 2, 128, T).transpose(2, 0, 1, 3)
        ).astype(ml_dtypes.float8_e4m3)
        sig_i = sig[sl]                                 # (2, 3C)
        sqk = sig_i[:, :2 * C]                          # (2, 2C)
        sq = ((sqk / W8SCALE).reshape(SPC, 12, 128)
              .transpose(2, 0, 1).reshape(128, SPC * 12))
        bq = ((qkv_b[None, :2 * C] * sqk).reshape(SPC, 12, 128)
              .transpose(2, 0, 1).reshape(128, SPC * 12))
        sigbq_i = np.ascontiguousarray(np.concatenate([sq, bq], axis=1))
        sigv = sig_i[:, 2 * C:]                         # (2, C)
        wv_sc = wvT[None, :, :] * sigv[:, None, :]      # (2, C, C)
        wv_pad = np.zeros((SPC, C, CP), np.float32)
        bvs_pad = np.zeros((SPC, CP), np.float32)
        for h in range(H):
            wv_pad[:, :, h * 65:h * 65 + 64] = wv_sc[:, :, h * 64:(h + 1) * 64]
            bvs_pad[:, h * 65:h * 65 + 64] = (
                qkv_b[None, 2 * C + h * 64: 2 * C + (h + 1) * 64]
                * sigv[:, h * 64:(h + 1) * 64])
            bvs_pad[:, h * 65 + 64] = 1.0
        in_maps.append({
            "xt": xt, "xt8": xt8, "wqk8": wqk8,
            "wv": np.ascontiguousarray(wv_pad).astype(ml_dtypes.bfloat16),
            "sigbq": sigbq_i,
            "bvs": np.ascontiguousarray(bvs_pad).astype(ml_dtypes.bfloat16),
            "pw": pw_bf, "pb": pb_arr,
        })
    return in_maps


_NC = None
LAST_RESULT = None


def kernel(x, label, alpha, qkv_w, qkv_b, proj_w, proj_b):
    global _NC, LAST_RESULT
    if _NC is None:
        _NC = build()
    in_maps = make_in_maps(x, label, alpha, qkv_w, qkv_b, proj_w, proj_b)
    res = run_bass_kernel_spmd(_NC, in_maps, core_ids=list(range(NCORES)))
    LAST_RESULT = res
    outs = []
    for i in range(NCORES):
        y = np.asarray(res.results[i]["out"])           # (C, T)
        outs.append(y.reshape(C, SPC, NT).transpose(1, 2, 0))
    return np.ascontiguousarray(np.concatenate(outs, axis=0), dtype=np.float32)



# revision 28
# speedup vs baseline: 1.0796x; 1.0741x over previous
"""AlphaWeightedHead Trainium2 kernel: per-sample sigmoid-gated QKV + MHA + proj.

Sharding: data-parallel over batch, 2 samples per core x 8 cores.
All device tensors use a feature-major ("transposed") layout so every matmul
reads its operands in natural orientation (no on-device transposes):

  x^T [c, t]  -> QKV^T [d, t] (Q/K) and V [t, hv]   (contraction over c)
  S^T [tk,tq] = K^T.T @ Q^T per head (contraction over hd)
  P^T = exp(S^T * scale)  (no max-subtract: |scores| < ~1, data-bounded)
  O^T_aug [65, tq] = [V | ones].T @ P^T  (row 64 = softmax denominator;
      the ones column is folded into the padded V weights host-side)
  Y^T [c_out, t] = pw^T.T @ (O^T / denom)

Perf structure (HW-validated on trn2):
  * QK projection GEMM runs in fp8e4m3 with MatmulPerfMode.DoubleRow:
    contraction 768 = 3 passes x (128 partitions x 2 k-tiles), halving the
    pass count vs bf16. Host pre-scales wqk8 by 16 into fp8's normal range
    and divides the f32 sigmoid gates by 16 to compensate exactly. Q/K/exp
    errors are damped by softmax normalization (rel err ~4e-3 total).
  * S^T stationaries are K=128 zero-padded per head (the other head's 64
    partitions hold zeros, memset once per pool slot). K=64 stationaries
    disable the PE's Fast Weight Load and cost ~2x; zero-padding keeps FWL
    on (-49 us/exec measured vs quadrant-packed K=64 tile_position pairs).
  * Softmax normalize multiplies the PV PSUM directly: reciprocal of the
    denominator row -> gpsimd partition_broadcast (SBUF) -> one DVE
    scalar_tensor_tensor per half-pair. No PSUM->SBUF staging copy and no
    PE broadcast matmul.
  * V GEMM / attention / proj stay bf16: fp8 on the V or proj path passes
    quantization error straight to the output (weighted means don't damp
    multiplicative operand noise), blowing the 2e-2 budget.

  * Pipeline schedule (engine queues are in-order FIFOs, so emission
    order = execution order per engine): the two samples are processed
    as separate pipelines, with each boundary interleaved as
    [V(s)-half0, QK+S^T(s, pair0), V(s)-half1, proj(s-1), pairs 1..5]
    — legal because pair0's PV reads only V columns 0:130. This keeps
    the activation engine fed across the cold start and the sample
    boundary while the previous projection drains under live
    attention. Weight staging (wv) is double-buffered so sample s+1's
    1.2MB DMA prefetches during sample s.
  * Confirmed-optimal by A/B (don't re-litigate): PSUM banks
    (mm 2 / st 4 / pv 2), exp at free-1024, LDWEIGHTS needs no
    stationary reuse, pt/qk/kpp pool depths.

V/PV/proj matmuls run in bf16 (PSUM accumulates fp32). Host precomputes
sigmoid(alpha[label]), pre-scales V weights/biases, and pads V with the
ones column per head (wv zero-col + bias 1.0 -> V-tile column of ones).
`build(reps=N)` unrolls the whole body N times in one NEFF: bench-only
amortization of dispatch noise; grading uses reps=1. Non-default build
flags preserve rejected A/B variants for reference.
"""

import sys

import numpy as np
import ml_dtypes
from contextlib import ExitStack

try:
    import concourse.bass as _probe  # noqa: F401
except ModuleNotFoundError:
    sys.path.insert(0, "/opt/trn_rl_repo")

import concourse.bass as bass
import concourse.bacc as bacc
import concourse.tile as tile
from concourse import mybir
from concourse.bass_utils import run_bass_kernel_spmd

B, NT, C, H, CLS = 16, 1024, 768, 12, 1000
HD = 64
NCORES = 8
SPC = B // NCORES          # samples per core = 2
T = SPC * NT               # tokens per core = 2048
NPAIR = H // 2             # 6 head pairs
CH = C // 128              # 6 contraction chunks
CP = H * (HD + 1)          # padded V width = 780 (65 per head)
SCALE = HD ** -0.5

F32 = mybir.dt.float32
BF16 = mybir.dt.bfloat16
FP8 = mybir.dt.float8e4
W8SCALE = 16.0  # host scales wqk8 by this so weights land in fp8 normals
QS = 16.0      # host scales Q/K by this into fp8 normals for the S GEMM
ADD = mybir.AluOpType.add
MULT = mybir.AluOpType.mult
EXP = mybir.ActivationFunctionType.Exp


def build(debug=False, phases="all", reps=1, st_kpad=True, epi_bcast=True,
          exp_fine=False, split_s=True, pvlag=1, wv2=True, qb=2,
          vhoist=True, vearly=True, vearly2=True, ptslack=False,
          jout=False, pv3=False, pvflip=True, tlag=0, gout=False,
          stb=2, mmb=2, pvb=2, s8=True):
    nc = bacc.Bacc("TRN2")
    xt = nc.declare_dram_parameter("xt", [C, T], BF16, isOutput=False)
    # fp8 DoubleRow operands for the QK projection GEMM: contraction 768 =
    # 3 passes x (128 partitions x 2 k-tiles). Layout [p, pass, ktile, n].
    xt8 = nc.declare_dram_parameter("xt8", [128, 3, 2, T], FP8, isOutput=False)
    wqk8 = nc.declare_dram_parameter("wqk8", [128, 3, 2, 2 * C], FP8,
                                     isOutput=False)
    # wv/pw are host-prearranged partition-major so stage-in DMAs are
    # contiguous per partition (the old "(c p) n -> p c n" rearrange cost
    # 768 descriptors and ~10us of cold-start serial DMA).
    wv = nc.declare_dram_parameter("wv", [SPC, 128, CH, CP], BF16,
                                   isOutput=False)
    sigbq = nc.declare_dram_parameter("sigbq", [128, 2 * SPC * 12], F32,
                                      isOutput=False)
    bvs = nc.declare_dram_parameter("bvs", [SPC, CP], BF16, isOutput=False)
    pw = nc.declare_dram_parameter("pw", [128, CH, C], BF16, isOutput=False)
    pb = nc.declare_dram_parameter("pb", [128, CH], F32, isOutput=False)
    out = nc.declare_dram_parameter("out", [C, T], F32, isOutput=True)
    if debug:
        dbg_v = nc.declare_dram_parameter("dbg_v", [128, 8 * CP], F32, isOutput=True)
        dbg_qk = nc.declare_dram_parameter("dbg_qk", [128, 2 * T], F32, isOutput=True)
        dbg_pt = nc.declare_dram_parameter("dbg_pt", [128, 2 * 8 * 512], F32, isOutput=True)
        dbg_st = nc.declare_dram_parameter("dbg_st", [128, 2 * 512], F32, isOutput=True)
        dbg_ob = nc.declare_dram_parameter("dbg_ob", [128, CH * T], F32, isOutput=True)

    with tile.TileContext(nc) as tc, ExitStack() as ctx:
        cpool = ctx.enter_context(tc.tile_pool(name="const", bufs=1))
        wvp = ctx.enter_context(tc.tile_pool(name="wvp", bufs=2 if wv2 else 1))
        qkp = ctx.enter_context(tc.tile_pool(name="qkp", bufs=qb))
        kpp = ctx.enter_context(tc.tile_pool(name="kpp", bufs=qb))
        ptp = ctx.enter_context(tc.tile_pool(name="ptp", bufs=2 * (pvlag + 1) + (2 if ptslack else 0)))
        stgp = ctx.enter_context(tc.tile_pool(name="stgp", bufs=4))
        yp = ctx.enter_context(tc.tile_pool(name="yp", bufs=3))
        mmps = ctx.enter_context(
            tc.tile_pool(name="mmps", bufs=1 if pv3 else mmb,
                         space=bass.MemorySpace.PSUM))
        stps = ctx.enter_context(
            tc.tile_pool(name="stps", bufs=4 if exp_fine else stb,
                         space=bass.MemorySpace.PSUM))
        pvps = ctx.enter_context(
            tc.tile_pool(name="pvps", bufs=3 if pv3 else pvb,
                         space=bass.MemorySpace.PSUM))

        # ---- resident tensors. Stage-in order/queues tuned for cold start:
        # the V GEMM needs wv[0]+xt first (SP queue); the QK path tensors
        # (xt8/wqk8) go on the Activation HWDGE queue so both halves land
        # in parallel; late-needed pw/pb trail.
        wv_sbs = {}
        wv_sbs[0] = wvp.tile([128, CH, CP], BF16, name="wv_sb")
        nc.sync.dma_start(wv_sbs[0][:], wv[0])
        xt_sb = cpool.tile([128, CH, T], BF16)
        for c in range(CH):
            nc.sync.dma_start(xt_sb[:, c, :], xt[c * 128:(c + 1) * 128, :])
        xt8_sb = cpool.tile([128, 3, 2, T], FP8)
        nc.scalar.dma_start(xt8_sb[:], xt8[:])
        wqk8_sb = cpool.tile([128, 3, 2, 2 * C], FP8)
        nc.scalar.dma_start(wqk8_sb[:], wqk8[:])
        sigbq_sb = cpool.tile([128, 2 * SPC * 12], F32)
        nc.scalar.dma_start(sigbq_sb[:], sigbq[:])
        bvs_sb = cpool.tile([128, SPC, CP], BF16)
        for s in range(SPC):
            nc.sync.dma_start(
                bvs_sb[:, s:s + 1, :], bvs[s:s + 1, :].partition_broadcast(128))
        pw_sb = cpool.tile([128, CH, C], BF16)
        nc.scalar.dma_start(pw_sb[:], pw[:])
        pb_sb = cpool.tile([128, CH], F32)
        nc.scalar.dma_start(pb_sb[:], pb[:])
        # pvflip: ob is t-chunk-major [tq-part? no: c-part, tchunk, cchunk(=pair),
        # 128 t] so each DMA-transposed [128tq, 128c] tile lands as one
        # contiguous 256B run per partition (XBAR transpose needs contiguous
        # SBUF dest runs).
        if pvflip:
            ob = cpool.tile([128, T // 128, CH, 128], BF16)
        else:
            ob = cpool.tile([128, CH, T], BF16)
        ones_t = cpool.tile([128, 64], BF16)
        nc.vector.memset(ones_t[0:1, :], 1.0)
        nc.vector.memset(ones_t[64:65, :], 1.0)
        vbs = [cpool.tile([128, 8, CP], BF16, tag=f"vb{s}", name=f"vb{s}")
               for s in range(SPC)]

        # ---- V phase: V_pad[t, 780] = x_s @ wv_pad + bvs_pad  (per sample)
        # wv_pad has a zero column per head; bvs_pad carries 1.0 there, so
        # the padded column becomes the all-ones denominator column.
        # (body emitted `reps` times for bench amortization; reps=1 for
        # grading)

        def emit_v_phase(vs=None, hvts=(0, 1)):
          for s in ([vs] if vs is not None else range(SPC)):
            if s in wv_sbs:
                wv_sb = wv_sbs[s]
            else:
                wv_sb = wvp.tile([128, CH, CP], BF16)
                nc.sync.dma_start(wv_sb[:], wv[s])
                wv_sbs[s] = wv_sb
            for tt in range(8):
                if jout and len(hvts) == 2:
                    pss = {hvt: mmps.tile([128, 512], F32, tag="mm",
                                          name=f"psv{hvt}")
                           for hvt in hvts}
                    for c in range(CH):
                        for hvt in hvts:
                            h0 = hvt * 512
                            hvn = 512 if hvt == 0 else CP - 512
                            nc.tensor.matmul(
                                pss[hvt][:, :hvn],
                                xt_sb[:, c,
                                      s * NT + tt * 128: s * NT + (tt + 1) * 128],
                                wv_sb[:, c, h0: h0 + hvn],
                                start=(c == 0), stop=(c == CH - 1),
                            )
                    for hvt in hvts:
                        h0 = hvt * 512
                        hvn = 512 if hvt == 0 else CP - 512
                        nc.vector.tensor_add(
                            vbs[s][:, tt, h0:h0 + hvn],
                            pss[hvt][:, :hvn],
                            bvs_sb[:, s, h0:h0 + hvn])
                    continue
                for hvt in hvts:
                    h0 = hvt * 512
                    hvn = 512 if hvt == 0 else CP - 512
                    ps = mmps.tile([128, 512], F32, tag="mm")
                    for c in range(CH):
                        nc.tensor.matmul(
                            ps[:, :hvn],
                            xt_sb[:, c, s * NT + tt * 128: s * NT + (tt + 1) * 128],
                            wv_sb[:, c, h0: h0 + hvn],
                            start=(c == 0), stop=(c == CH - 1),
                        )
                    nc.vector.tensor_add(
                        vbs[s][:, tt, h0:h0 + hvn],
                        ps[:, :hvn],
                        bvs_sb[:, s, h0:h0 + hvn])
                    if debug and s == 0:
                        dv = yp.tile([128, 512], F32, tag="dbgv", name="dv")
                        nc.vector.tensor_copy(dv[:, :hvn], vbs[s][:, tt, h0:h0 + hvn])
                        nc.sync.dma_start(
                            dbg_v[:, tt * CP + h0: tt * CP + h0 + hvn], dv[:, :hvn])

        # ---- PV + epilogue, one iteration behind S^T/exp (keeps ACT fed)
        pending = []
        tpend = []  # deferred transpose DMAs: (tt, p, otok)

        def flush_tp(keep=0):
            while len(tpend) > keep:
                ftt, fp, fotok = tpend.pop(0)
                nc.sync.dma_start(ob[:, ftt, fp, :], fotok[:], transpose=True)

        subq = []  # pending PV sub-chunks: (p, s, tq, pt, sub)

        def emit_pv_sub():
            # P^T-stationary PV: out O[tq128, 65] uses all 128 PSUM rows
            # (vs 65 with V-stationary), halving PV's PE cycles. The
            # normalized [tq, c-pair] tile is transposed into ob by the
            # DMA XBAR (14ns per 16x128 tile, zero PE cost). One sub-chunk
            # (16 matmuls) is emitted per S-loop tk2 slot so PV fills the
            # PE idle while ACT drains the st ping-pong.
            if not subq:
                return
            p, s, tq, pt, sub = subq.pop(0)
            q0 = sub * 128
            tt = (s * NT + tq * 512) // 128 + sub
            pv = pvps.tile([128, 2, 65], F32, tag="pv", name="pv")
            for hh in range(2):
                for chk in range(8):
                    nc.tensor.matmul(
                        pv[:, hh, :],
                        pt[hh][:, chk, q0:q0 + 128],
                        vbs[s][:, chk, (2 * p + hh) * 65:
                               (2 * p + hh) * 65 + 65],
                        start=(chk == 0), stop=(chk == 7),
                    )
            rsb = stgp.tile([128, 2], F32, tag="rsb", name="rsb")
            otok = stgp.tile([128, 128], BF16, tag="otok", name="otok",
                             bufs=4 * (tlag + 1) + 1)
            for hh in range(2):
                nc.vector.reciprocal(
                    rsb[:, hh:hh + 1], pv[:, hh, 64:65])
                nc.vector.tensor_scalar(
                    otok[:, hh * 64:(hh + 1) * 64], pv[:, hh, 0:64],
                    rsb[:, hh:hh + 1], None, MULT)
            tpend.append((tt, p, otok))
            flush_tp(keep=4 * tlag)

        def emit_pv_flip(item):
            # enqueue only; subs are drained one per S-loop tk2 slot
            p, s, tq, pt = item
            for sub in range(4):
                subq.append((p, s, tq, pt, sub))

        def drain_pv():
            while pending:
                emit_pv(pending.pop(0))
            while subq:
                emit_pv_sub()

        def emit_pv(item):
            if pvflip:
                emit_pv_flip(item)
                return
            p, s, tq, pt = item
            for hh in range(2):
                h = 2 * p + hh
                pv = pvps.tile([128, 512], F32, tag="pv", name="pv")
                for chk in range(8):
                    nc.tensor.matmul(
                        pv[0:65, :],
                        vbs[s][:, chk, h * 65: h * 65 + 65],
                        pt[hh][:, chk, :],
                        start=(chk == 0), stop=(chk == 7),
                    )
                if epi_bcast:
                    rsb = stgp.tile([1, 512], BF16, tag="rsb", name="rsb")
                    with nc.allow_low_precision(reason="softmax denom bf16"):
                        nc.vector.reciprocal(rsb[:], pv[64:65, :])
                    mul2 = stgp.tile([64, 512], BF16, tag="rbb", name="rbb")
                    nc.gpsimd.partition_broadcast(mul2[:], rsb[:], channels=64)
                    src = pv
                else:
                    stg = stgp.tile([128, 512], BF16, tag="stg", name="stg")
                    nc.vector.tensor_copy(stg[0:65, :], pv[0:65, :])
                    with nc.allow_low_precision(reason="softmax denom bf16"):
                        nc.vector.reciprocal(stg[64:65, :], stg[64:65, :])
                    rb = pvps.tile([128, 512], F32, tag="pv", name="rb")
                    nc.tensor.matmul(
                        rb[0:64, :],
                        ones_t[64:65, :],
                        stg[64:65, :],
                        start=True, stop=True,
                        tile_position=(64, 0),
                    )
                    mul2 = rb
                    src = stg
                if hh == 0:
                    nc.vector.scalar_tensor_tensor(
                        ob[0:64, p, s * NT + tq * 512: s * NT + (tq + 1) * 512],
                        src[0:64, :], 0.0, mul2[0:64, :],
                        mybir.AluOpType.bypass, MULT)
                else:
                    stn = stgp.tile([64, 512], BF16, tag="stn", name="stn")
                    nc.vector.scalar_tensor_tensor(
                        stn[:], src[0:64, :], 0.0, mul2[0:64, :],
                        mybir.AluOpType.bypass, MULT)
                    nc.sync.dma_start(
                        ob[64:128, p,
                           s * NT + tq * 512: s * NT + (tq + 1) * 512],
                        stn[:])

        # ---- head-pair loop: QKV(Q,K) -> S^T -> exp -> PV -> normalize
        # S^T stationaries are K=128 zero-padded per head (the other head's
        # 64 partitions hold zeros) so FWL stays enabled; the moving Q
        # streams both heads' rows and the zeros mask the wrong head.
        # Zero halves are memset once per pool slot and never rewritten.
        TW = NT if split_s else T
        ESC = SCALE / (QS * QS) if s8 else SCALE
        if s8:
            # fp8 DoubleRow S: one [128, 2, TW] fp8 tile per operand.
            # Head-even lives at partitions 0:64 / ktile 0, head-odd at
            # 64:128 / ktile 1; the complementary ktile halves are zero
            # (memset once per pool slot) so the DR 2-ktile sum only sees
            # one head per matmul, mirroring the bf16 zero-pad trick.
            # Host scales Q,K by 16 into fp8 normals; exp absorbs 1/256.
            for _i in range(qb):
                q8 = qkp.tile([128, 2, TW], FP8, tag="q8", name="q8")
                k8 = kpp.tile([128, 2, TW], FP8, tag="k8", name="k8")
                for t8 in (q8, k8):
                    nc.vector.memset(t8[0:64, 1, :], 0.0)
                    nc.vector.memset(t8[64:128, 0, :], 0.0)
        elif st_kpad:
            kp_init = []
            for _i in range(2):
                ke = kpp.tile([128, TW], BF16, tag="ke", name="ke")
                ko = kpp.tile([128, TW], BF16, tag="ko", name="ko")
                nc.vector.memset(ke[64:128, :], 0.0)
                nc.vector.memset(ko[0:64, :], 0.0)
                kp_init.append((ke, ko))

        def emit_heads(s_sel=None, pairs=None):
         nos = phases in ("v", "vqk", "qkonly")
         slist = ([] if nos else [s_sel]) if split_s else \
             list(range(0 if nos else SPC))
         if pairs is None:
             pairs = range(NPAIR if phases != "v" else 0)
         for p in pairs:
            if s8:
                q8 = qkp.tile([128, 2, TW], FP8, tag="q8", name="q8")
                k8 = kpp.tile([128, 2, TW], FP8, tag="k8", name="k8")
            else:
                qk_t = qkp.tile([128, 1 if st_kpad else 2, TW], BF16)
            if not s8 and st_kpad:
                ke = kpp.tile([128, TW], BF16, tag="ke", name="ke")
                ko = kpp.tile([128, TW], BF16, tag="ko", name="ko")
            for qk in range(2):
                d0 = qk * C + p * 128
                nlist = list(range(2 if split_s else 4))
                pss = {}
                if jout:
                    for n in nlist:
                        pss[n] = mmps.tile([128, 512], F32, tag="mm",
                                           name=f"psq{n}")
                    for j in range(3):
                        for n in nlist:
                            g = (2 * s_sel + n) if split_s else n
                            nc.tensor.matmul(
                                pss[n][:],
                                wqk8_sb[:, j, :, d0:d0 + 128],
                                xt8_sb[:, j, :, g * 512:(g + 1) * 512],
                                start=(j == 0), stop=(j == 2),
                                perf_mode=mybir.MatmulPerfMode.DoubleRow,
                            )
                for n in nlist:
                    g = (2 * s_sel + n) if split_s else n
                    if jout:
                        ps = pss[n]
                    else:
                        ps = mmps.tile([128, 512], F32, tag="mm")
                        for j in range(3):
                            nc.tensor.matmul(
                                ps[:],
                                wqk8_sb[:, j, :, d0:d0 + 128],
                                xt8_sb[:, j, :, g * 512:(g + 1) * 512],
                                start=(j == 0), stop=(j == 2),
                                perf_mode=mybir.MatmulPerfMode.DoubleRow,
                            )
                    sj = s_sel if split_s else n // 2
                    j = sj * 12 + qk * 6 + p
                    if s8:
                        t8 = k8 if qk else q8
                        with nc.allow_low_precision(reason="fp8 S operands"):
                            nc.vector.tensor_scalar(
                                t8[0:64, 0, n * 512:(n + 1) * 512],
                                ps[0:64, :],
                                sigbq_sb[0:64, j:j + 1],
                                sigbq_sb[0:64, 24 + j:24 + j + 1],
                                MULT, ADD)
                            nc.vector.tensor_scalar(
                                t8[64:128, 1, n * 512:(n + 1) * 512],
                                ps[64:128, :],
                                sigbq_sb[64:128, j:j + 1],
                                sigbq_sb[64:128, 24 + j:24 + j + 1],
                                MULT, ADD)
                    elif st_kpad and qk == 1:
                        nc.vector.tensor_scalar(
                            ke[0:64, n * 512:(n + 1) * 512], ps[0:64, :],
                            sigbq_sb[0:64, j:j + 1],
                            sigbq_sb[0:64, 24 + j:24 + j + 1],
                            MULT, ADD)
                        nc.vector.tensor_scalar(
                            ko[64:128, n * 512:(n + 1) * 512], ps[64:128, :],
                            sigbq_sb[64:128, j:j + 1],
                            sigbq_sb[64:128, 24 + j:24 + j + 1],
                            MULT, ADD)
                    else:
                        nc.vector.tensor_scalar(
                            qk_t[:, qk, n * 512:(n + 1) * 512], ps[:],
                            sigbq_sb[:, j:j + 1], sigbq_sb[:, 24 + j:24 + j + 1],
                            MULT, ADD)
                    if debug and p == 0 and not st_kpad:
                        dq = yp.tile([128, 512], F32, tag="dbgq", name="dq")
                        nc.vector.tensor_copy(
                            dq[:], qk_t[:, qk, n * 512:(n + 1) * 512])
                        nc.sync.dma_start(
                            dbg_qk[:, qk * T + n * 512: qk * T + (n + 1) * 512],
                            dq[:])

            for s in slist:
                base = 0 if split_s else s * NT
                for tq in range(2):
                    pt = [ptp.tile([128, 8, 512], BF16, tag="pt", name=f"pt{_h}")
                          for _h in range(2)]

                    def st_mm(dst, tk, hh):
                        lo = hh * 64
                        if s8:
                            nc.tensor.matmul(
                                dst,
                                k8[lo:lo + 64, :,
                                   base + tk * 128: base + (tk + 1) * 128],
                                q8[lo:lo + 64, :,
                                   base + tq * 512: base + (tq + 1) * 512],
                                start=True, stop=True,
                                perf_mode=mybir.MatmulPerfMode.DoubleRow,
                            )
                        elif st_kpad:
                            kt = ko if hh else ke
                            nc.tensor.matmul(
                                dst,
                                kt[:, base + tk * 128:
                                   base + (tk + 1) * 128],
                                qk_t[:, 0,
                                     base + tq * 512:
                                     base + (tq + 1) * 512],
                                start=True, stop=True,
                            )
                        else:
                            nc.tensor.matmul(
                                dst,
                                qk_t[lo:lo + 64, 1,
                                     base + tk * 128:
                                     base + (tk + 1) * 128],
                                qk_t[lo:lo + 64, 0,
                                     base + tq * 512:
                                     base + (tq + 1) * 512],
                                start=True, stop=True,
                                tile_position=(lo, 0),
                            )

                    if exp_fine:
                        for tk in range(8):
                            stf = [stps.tile([128, 512], F32, tag="st",
                                             name=f"stf{_h}")
                                   for _h in range(2)]
                            for hh in range(2):
                                st_mm(stf[hh][:], tk, hh)
                            for hh in range(2):
                                nc.scalar.activation(
                                    pt[hh][:, tk, :],
                                    stf[hh][:], EXP, scale=ESC)
                    else:
                        for tk2 in range(4):
                            st2 = [stps.tile([128, 2, 512], F32, tag="st",
                                             name=f"st{_h}")
                                   for _h in range(2)]
                            for sub in range(2):
                                tk = 2 * tk2 + sub
                                for hh in range(2):
                                    st_mm(st2[hh][:, sub, :], tk, hh)
                            for hh in range(2):
                                nc.scalar.activation(
                                    pt[hh][:, 2 * tk2:2 * tk2 + 2, :],
                                    st2[hh][:], EXP, scale=ESC)
                            if pvflip:
                                emit_pv_sub()
                    if phases != "stexp":
                        pending.append((p, s, tq, pt))
                        if len(pending) > pvlag:
                            emit_pv(pending.pop(0))
        def ob_mov(c, n):
            # O^T moving slice [128c, 512t] for proj: 4 t-chunks of 128
            if pvflip:
                return ob[:, 4 * n:4 * n + 4, c, :]
            return ob[:, c, n * 512:(n + 1) * 512]

        def emit_tail(nlist):
          if debug and phases == "all":
            for c in range(CH):
                for n in nlist:
                    do = yp.tile([128, 512], F32, tag="dbgo", name="do")
                    nc.vector.tensor_copy(do[:], ob_mov(c, n))
                    nc.sync.dma_start(
                        dbg_ob[:, c * T + n * 512: c * T + (n + 1) * 512], do[:])
          # ---- proj: Y^T = pw^T.T @ O^T + pb
          for m in range(CH if phases == "all" else 0):
            for n in nlist:
                ps = mmps.tile([128, 512], F32, tag="mm")
                for c in range(CH):
                    nc.tensor.matmul(
                        ps[:],
                        pw_sb[:, c, m * 128:(m + 1) * 128],
                        ob_mov(c, n),
                        start=(c == 0), stop=(c == CH - 1),
                    )
                y_t = yp.tile([128, 512], F32)
                nc.vector.tensor_scalar(y_t[:], ps[:], pb_sb[:, m:m + 1], None, ADD)
                (nc.gpsimd if gout else nc.sync).dma_start(
                    out[m * 128:(m + 1) * 128, n * 512:(n + 1) * 512], y_t[:])

        for _rep in range(reps):
            if split_s and vearly2 and phases == "all":
                # interleave each sample's V/attention start with the
                # previous sample's projection to keep ACT fed across
                # the boundary (PE queues are in-order FIFOs)
                for s in range(SPC):
                    emit_v_phase(s, hvts=(0,))
                    emit_heads(s, pairs=[0])
                    emit_v_phase(s, hvts=(1,))
                    if s > 0:
                        emit_tail([2 * (s - 1), 2 * (s - 1) + 1])
                    emit_heads(s, pairs=list(range(1, NPAIR)))
                    drain_pv()
                    flush_tp()
                emit_tail([2 * (SPC - 1), 2 * (SPC - 1) + 1])
            elif split_s:
                for s in range(SPC):
                    if s == 0:
                        if vearly and phases == "all":
                            emit_v_phase(0, hvts=(0,))
                            emit_heads(0, pairs=[0])
                            emit_v_phase(0, hvts=(1,))
                            emit_heads(0, pairs=list(range(1, NPAIR)))
                        else:
                            emit_v_phase(0)
                            emit_heads(0)
                    else:
                        emit_heads(s)
                    drain_pv()
                    flush_tp()
                    if vhoist and s + 1 < SPC:
                        emit_v_phase(s + 1)
                    emit_tail([2 * s, 2 * s + 1])
                    if not vhoist and s + 1 < SPC:
                        emit_v_phase(s + 1)
            else:
                emit_v_phase()
            if not split_s:
                emit_heads()
                drain_pv()
                flush_tp()
                emit_tail([0, 1, 2, 3])
    nc.compile()
    return nc


def make_in_maps(x, label, alpha, qkv_w, qkv_b, proj_w, proj_b):
    x = np.asarray(x, np.float32)
    label = np.asarray(label)
    alpha = np.asarray(alpha, np.float32)
    qkv_w = np.asarray(qkv_w, np.float32)
    qkv_b = np.asarray(qkv_b, np.float32)
    proj_w = np.asarray(proj_w, np.float32)
    proj_b = np.asarray(proj_b, np.float32)

    sig = 1.0 / (1.0 + np.exp(-alpha[label]))          # (B, 3C) f32
    wqkT = np.ascontiguousarray(qkv_w[:2 * C].T)        # (C, 2C) f32
    # fp8 DoubleRow layout [p, pass, ktile, d], weights pre-scaled by
    # W8SCALE into fp8's normal range (compensated in the sig scalars)
    wqk8 = np.ascontiguousarray(
        (wqkT * W8SCALE).reshape(3, 2, 128, 2 * C).transpose(2, 0, 1, 3)
    ).astype(ml_dtypes.float8_e4m3)
    wvT = np.ascontiguousarray(qkv_w[2 * C:].T)         # (C, C) f32
    pw_bf = np.ascontiguousarray(proj_w.T).astype(ml_dtypes.bfloat16)
    pb_arr = np.ascontiguousarray(proj_b.reshape(CH, 128).T)

    in_maps = []
    for i in range(NCORES):
        sl = slice(SPC * i, SPC * (i + 1))
        xs = x[sl]                                      # (2, NT, C)
        xt_f = xs.transpose(2, 0, 1).reshape(C, T)      # (C, T) f32
        xt = np.ascontiguousarray(xt_f).astype(ml_dtypes.bfloat16)
        xt8 = np.ascontiguousarray(
            xt_f.reshape(3, 2, 128, T).transpose(2, 0, 1, 3)
        ).astype(ml_dtypes.float8_e4m3)
        sig_i = sig[sl]                                 # (2, 3C)
        sqk = sig_i[:, :2 * C]                          # (2, 2C)
        # gate scalars fold two fp8 scalings: /W8SCALE undoes the wqk8
        # pre-scale from the QK GEMM; *QS lands Q,K in fp8 normals for the
        # DoubleRow S GEMM (exp absorbs 1/QS^2)
        sq = ((sqk * (QS / W8SCALE)).reshape(SPC, 12, 128)
              .transpose(2, 0, 1).reshape(128, SPC * 12))
        bq = ((qkv_b[None, :2 * C] * sqk * QS).reshape(SPC, 12, 128)
              .transpose(2, 0, 1).reshape(128, SPC * 12))
        sigbq_i = np.ascontiguousarray(np.concatenate([sq, bq], axis=1))
        sigv = sig_i[:, 2 * C:]                         # (2, C)
        wv_sc = wvT[None, :, :] * sigv[:, None, :]      # (2, C, C)
        wv_pad = np.zeros((SPC, C, CP), np.float32)
        bvs_pad = np.zeros((SPC, CP), np.float32)
        for h in range(H):
            wv_pad[:, :, h * 65:h * 65 + 64] = wv_sc[:, :, h * 64:(h + 1) * 64]
            bvs_pad[:, h * 65:h * 65 + 64] = (
                qkv_b[None, 2 * C + h * 64: 2 * C + (h + 1) * 64]
                * sigv[:, h * 64:(h + 1) * 64])
            bvs_pad[:, h * 65 + 64] = 1.0
        wv_pm = wv_pad.reshape(SPC, CH, 128, CP).transpose(0, 2, 1, 3)
        in_maps.append({
            "xt": xt, "xt8": xt8, "wqk8": wqk8,
            "wv": np.ascontiguousarray(wv_pm).astype(ml_dtypes.bfloat16),
            "sigbq": sigbq_i,
            "bvs": np.ascontiguousarray(bvs_pad).astype(ml_dtypes.bfloat16),
            "pw": np.ascontiguousarray(
                pw_bf.reshape(CH, 128, C).transpose(1, 0, 2)),
            "pb": pb_arr,
        })
    return in_maps


_NC = None
LAST_RESULT = None


def kernel(x, label, alpha, qkv_w, qkv_b, proj_w, proj_b):
    global _NC, LAST_RESULT
    if _NC is None:
        _NC = build()
    in_maps = make_in_maps(x, label, alpha, qkv_w, qkv_b, proj_w, proj_b)
    res = run_bass_kernel_spmd(_NC, in_maps, core_ids=list(range(NCORES)))
    LAST_RESULT = res
    outs = []
    for i in range(NCORES):
        y = np.asarray(res.results[i]["out"])           # (C, T)
        outs.append(y.reshape(C, SPC, NT).transpose(1, 2, 0))
    return np.ascontiguousarray(np.concatenate(outs, axis=0), dtype=np.float32)

